# revision 3
# baseline (speedup 1.0000x reference)
"""Trainium2 Bass kernel for NMIL (nested multiple-instance learning) architecture.

Computation (see reference): gated-attention MIL logits per instance, per-region
(segment) softmax + attention-pooled region embeddings, second-level attention
over regions, classifier MLP.

Distribution: instance dim N=131072 sharded across 8 cores (16384 each).
Host pre-transposes/casts the feature shard to f16 twice (natural [Nl,512] and
transposed [512,Nl]) so the device never transposes the big tensor; segment
softmax/sum partials are AllReduced across cores in-kernel; every core
redundantly computes the tiny level-2 tail; host takes core 0's result.

f16 is used for the large GEMM operands (X, Wv, Wu, one-hot*e); all
accumulation is f32 in PSUM. Verified end-to-end rel err ~2e-4 vs f32 ref.
"""

import numpy as np

import concourse.bacc as bacc
import concourse.mybir as mybir
import concourse.tile as tile
from concourse.bass_utils import run_bass_kernel_spmd

dt = mybir.dt
AF = mybir.ActivationFunctionType
OP = mybir.AluOpType

NCORES = 8
N, L, D, R = 131072, 512, 128, 64
NL = N // NCORES           # 16384 instances per core
NT = NL // 128             # 128 chunks of 128 instances
MACRO = 4                  # chunks per macro-tile (512 instances)
NMACRO = NT // MACRO       # 32

_RUN_KWARGS: dict = {}     # test.py may set dict(trace=True, ...)
_CACHE: dict = {}


def _build():
    nc = bacc.Bacc("TRN2", target_bir_lowering=False, debug=False,
                   num_devices=NCORES)

    def din(name, shape, dtype):
        return nc.dram_tensor(name, shape, dtype, kind="ExternalInput").ap()

    xt = din("xt", [L, NL], dt.float16)          # transposed feature shard
    xn = din("xn", [NL, L], dt.float16)          # natural feature shard
    segt = din("segt", [128, NT], dt.float32)    # tiled seg ids: [p,j]=seg[j*128+p]
    iota = din("iota", [128, 128], dt.float32)   # each row = 0..127
    cntb = din("cntb", [128, R], dt.float32)     # global counts bcast over rows
    wv = din("wv", [L, D], dt.float16)
    wu = din("wu", [L, D], dt.float16)
    aw16 = din("aw16", [D, 1], dt.float16)
    avb = din("avb", [D, 1], dt.float32)
    aub = din("aub", [D, 1], dt.float32)
    awb = din("awb", [128, 1], dt.float32)       # aw_b replicated
    ones16 = din("ones16", [128, 1], dt.float16)
    ident = din("ident", [128, 128], dt.float32)
    c1w = din("c1w", [L, 256], dt.float32)
    c1b = din("c1b", [128, 2], dt.float32)
    c2w = din("c2w", [256, D], dt.float32)
    c2b = din("c2b", [128, 1], dt.float32)
    c3w = din("c3w", [D, 2], dt.float32)
    c3b = din("c3b", [2, 1], dt.float32)

    patch_t = nc.dram_tensor("patch_t", [128, NT], dt.float32,
                             kind="ExternalOutput").ap()
    glob = nc.dram_tensor("glob", [2, 1], dt.float32,
                          kind="ExternalOutput").ap()

    with tile.TileContext(nc) as tc:
        with (
            tc.tile_pool(name="const", bufs=1) as cp,
            tc.tile_pool(name="res", bufs=1) as rp,
            tc.tile_pool(name="dram", bufs=1, space="DRAM") as dp,
        ):
            # ---- load constants ----
            iota_sb = cp.tile([128, 128], dt.float32)
            nc.sync.dma_start(iota_sb[:], iota[:])
            cnt_sb = cp.tile([128, R], dt.float32)
            nc.sync.dma_start(cnt_sb[:], cntb[:])
            wv_sb = cp.tile([128, 4 * D], dt.float16)
            wu_sb = cp.tile([128, 4 * D], dt.float16)
            for fc in range(4):
                nc.sync.dma_start(wv_sb[:, fc * D:(fc + 1) * D],
                                  wv[fc * 128:(fc + 1) * 128, :])
                nc.sync.dma_start(wu_sb[:, fc * D:(fc + 1) * D],
                                  wu[fc * 128:(fc + 1) * 128, :])
            aw_sb = cp.tile([D, 1], dt.float16)
            nc.sync.dma_start(aw_sb[:], aw16[:])
            avb_sb = cp.tile([D, 1], dt.float32)
            nc.sync.dma_start(avb_sb[:], avb[:])
            aub_sb = cp.tile([D, 1], dt.float32)
            nc.sync.dma_start(aub_sb[:], aub[:])
            awb_sb = cp.tile([128, 1], dt.float32)
            nc.sync.dma_start(awb_sb[:], awb[:])
            ones_sb = cp.tile([128, 1], dt.float16)
            nc.sync.dma_start(ones_sb[:], ones16[:])
            ident_sb = cp.tile([128, 128], dt.float32)
            nc.sync.dma_start(ident_sb[:], ident[:])
            c1w_sb = cp.tile([128, 4 * 256], dt.float32)
            for fc in range(4):
                nc.sync.dma_start(c1w_sb[:, fc * 256:(fc + 1) * 256],
                                  c1w[fc * 128:(fc + 1) * 128, :])
            c1b_sb = cp.tile([128, 2], dt.float32)
            nc.sync.dma_start(c1b_sb[:], c1b[:])
            c2w_sb = cp.tile([128, 2 * D], dt.float32)
            for mc in range(2):
                nc.sync.dma_start(c2w_sb[:, mc * D:(mc + 1) * D],
                                  c2w[mc * 128:(mc + 1) * 128, :])
            c2b_sb = cp.tile([128, 1], dt.float32)
            nc.sync.dma_start(c2b_sb[:], c2b[:])
            c3w_sb = cp.tile([D, 2], dt.float32)
            nc.sync.dma_start(c3w_sb[:], c3w[:])
            c3b_sb = cp.tile([2, 1], dt.float32)
            nc.sync.dma_start(c3b_sb[:], c3b[:])

            # ---- resident buffers ----
            seg_all = rp.tile([128, NT], dt.float32)
            nc.sync.dma_start(seg_all[:], segt[:])
            w_all = rp.tile([128, NT], dt.float32)
            e_all = rp.tile([128, NT], dt.float32)
            zi_all = rp.tile([128, NT], dt.float32)
            cg_all = rp.tile([128, NT], dt.float32)
            patch_sb = rp.tile([128, NT], dt.float32)
            rem_un = rp.tile([R, L + 1], dt.float32)   # unnormed region emb | z
            rem_g = rp.tile([R, L + 1], dt.float32)    # after AllReduce
            rem_sb = rp.tile([R, L], dt.float32)       # normalized region emb
            z_row = rp.tile([1, R], dt.float32)
            z_bc = rp.tile([128, R], dt.float32)

            bounce_in = dp.tile([R, L + 1], dt.float32)
            bounce_out = dp.tile([R, L + 1], dt.float32)

            # ================= phase 1: per-instance MIL + partial segsums ===
            with (
                tc.tile_pool(name="xin", bufs=3) as xp,
                tc.tile_pool(name="mid", bufs=2) as mp,
                tc.tile_pool(name="oep", bufs=3) as op_,
                tc.tile_pool(name="psm", bufs=2, space="PSUM") as psm,
                tc.tile_pool(name="psw", bufs=2, space="PSUM") as psw,
                tc.tile_pool(name="psacc", bufs=1, space="PSUM") as psa,
            ):
                rem_ps = psa.tile([128, L], dt.float32, tag="rem")
                z_ps = psa.tile([128, 1], dt.float32, tag="z")
                for m in range(NMACRO):
                    i0 = m * 512
                    xtm = xp.tile([128, 4 * 512], dt.float16, tag="xt")
                    xnm = xp.tile([128, 4 * 512], dt.float16, tag="xn")
                    for fc in range(4):
                        nc.sync.dma_start(
                            xtm[:, fc * 512:(fc + 1) * 512],
                            xt[fc * 128:(fc + 1) * 128, i0:i0 + 512])
                    for t in range(4):
                        nc.sync.dma_start(
                            xnm[:, t * 512:(t + 1) * 512],
                            xn[i0 + t * 128:i0 + (t + 1) * 128, :])

                    av_ps = psm.tile([128, 512], dt.float32, tag="av")
                    au_ps = psm.tile([128, 512], dt.float32, tag="au")
                    for fc in range(4):
                        nc.tensor.matmul(
                            av_ps[:], wv_sb[:, fc * D:(fc + 1) * D],
                            xtm[:, fc * 512:(fc + 1) * 512],
                            start=(fc == 0), stop=(fc == 3))
                    for fc in range(4):
                        nc.tensor.matmul(
                            au_ps[:], wu_sb[:, fc * D:(fc + 1) * D],
                            xtm[:, fc * 512:(fc + 1) * 512],
                            start=(fc == 0), stop=(fc == 3))

                    av_sb = mp.tile([128, 512], dt.float32, tag="avs")
                    au_sb = mp.tile([128, 512], dt.float32, tag="aus")
                    nc.scalar.activation(av_sb[:], av_ps[:], AF.Tanh,
                                         bias=avb_sb[:])
                    nc.scalar.activation(au_sb[:], au_ps[:], AF.Sigmoid,
                                         bias=aub_sb[:])
                    h16 = mp.tile([128, 512], dt.float16, tag="h")
                    nc.vector.tensor_mul(h16[:], av_sb[:], au_sb[:])

                    w_ps = psw.tile([128, 4], dt.float32, tag="w")
                    for t in range(4):
                        nc.tensor.matmul(w_ps[:, t:t + 1],
                                         h16[:, t * 128:(t + 1) * 128],
                                         aw_sb[:], start=True, stop=True,
                                         skip_group_check=True)
                    c4 = slice(m * 4, (m + 1) * 4)
                    nc.scalar.activation(e_all[:, c4], w_ps[:], AF.Exp,
                                         bias=awb_sb[:])
                    nc.vector.tensor_scalar_add(w_all[:, c4], w_ps[:],
                                                awb_sb[:])

                    for t in range(4):
                        j = m * 4 + t
                        oe = op_.tile([128, 128], dt.float16, tag="oe")
                        nc.vector.tensor_scalar(
                            oe[:], iota_sb[:], seg_all[:, j:j + 1],
                            e_all[:, j:j + 1], op0=OP.is_equal, op1=OP.mult)
                        first = (m == 0 and t == 0)
                        last = (m == NMACRO - 1 and t == 3)
                        nc.tensor.matmul(rem_ps[:], oe[:],
                                         xnm[:, t * 512:(t + 1) * 512],
                                         start=first, stop=last,
                                         skip_group_check=True)
                        nc.tensor.matmul(z_ps[:], oe[:], ones_sb[:],
                                         start=first, stop=last,
                                         skip_group_check=True)

                nc.scalar.activation(rem_un[:, 0:L], rem_ps[0:R, :], AF.Copy)
                nc.scalar.activation(rem_un[:, L:L + 1], z_ps[0:R, :], AF.Copy)

            # ================= AllReduce partials across the 8 cores =========
            nc.sync.dma_start(bounce_in[:], rem_un[:])
            nc.gpsimd.collective_compute(
                "AllReduce", OP.add,
                replica_groups=[list(range(NCORES))],
                ins=[bounce_in.opt()], outs=[bounce_out.opt()])
            nc.sync.dma_start(rem_g[:], bounce_out[:])

            # ================= phase 2: per-instance softmax + patch out =====
            with (
                tc.tile_pool(name="p2s", bufs=4) as p2s,
                tc.tile_pool(name="p2p", bufs=2, space="PSUM") as p2p,
            ):
                # z as a row, broadcast to 128 partitions via K=1 matmul
                nc.sync.dma_start(
                    z_row[0:1, :],
                    bounce_out[:, L:L + 1].rearrange("a b -> b a"))
                onecol = p2s.tile([1, 128], dt.float32, tag="onec")
                nc.gpsimd.memset(onecol[:], 1.0)
                zbc_ps = p2p.tile([128, R], dt.float32, tag="zbc")
                nc.tensor.matmul(zbc_ps[:], onecol[:], z_row[0:1, :],
                                 start=True, stop=True, skip_group_check=True)
                nc.vector.tensor_copy(z_bc[:], zbc_ps[:])

                for j in range(NT):
                    scratch = p2s.tile([128, R], dt.float32, tag="scr")
                    nc.vector.scalar_tensor_tensor(
                        scratch[:], iota_sb[:, 0:R], seg_all[:, j:j + 1],
                        z_bc[:], op0=OP.is_equal, op1=OP.mult,
                        accum_out=zi_all[:, j:j + 1])
                    scratch2 = p2s.tile([128, R], dt.float32, tag="scr2")
                    nc.vector.scalar_tensor_tensor(
                        scratch2[:], iota_sb[:, 0:R], seg_all[:, j:j + 1],
                        cnt_sb[:], op0=OP.is_equal, op1=OP.mult,
                        accum_out=cg_all[:, j:j + 1])

                rz_all = p2s.tile([128, NT], dt.float32, tag="rz")
                nc.vector.reciprocal(rz_all[:], zi_all[:])
                sm_all = p2s.tile([128, NT], dt.float32, tag="sm")
                nc.vector.tensor_mul(sm_all[:], e_all[:], rz_all[:])
                mask_all = p2s.tile([128, NT], dt.int32, tag="mask")
                nc.vector.tensor_scalar(mask_all[:], cg_all[:], 1.0, None,
                                        op0=OP.is_equal)
                nc.vector.select(patch_sb[:], mask_all[:], w_all[:],
                                 sm_all[:])
                nc.sync.dma_start(patch_t[:], patch_sb[:])

            # ================= phase 3: level-2 attention + classifier =======
            with (
                tc.tile_pool(name="p3s", bufs=1) as p3s,
                tc.tile_pool(name="p3p", bufs=4, space="PSUM") as p3p,
            ):
                # rem = rem_g[:, :L] / z
                rzg = p3s.tile([R, 1], dt.float32, tag="rzg")
                nc.vector.reciprocal(rzg[:], rem_g[:, L:L + 1])
                nc.vector.tensor_scalar_mul(rem_sb[:], rem_g[:, 0:L], rzg[:])

                # remT via PE transposes (f32 in, cast to f16 on copy-out)
                remT_ps = p3p.tile([128, 4 * R], dt.float32, tag="p3")
                for fc in range(4):
                    nc.tensor.matmul(remT_ps[:, fc * R:(fc + 1) * R],
                                     rem_sb[:, fc * 128:(fc + 1) * 128],
                                     ident_sb[0:R, 0:R], is_transpose=True,
                                     start=True, stop=True,
                                     skip_group_check=True)
                remT16 = p3s.tile([128, 4 * R], dt.float16, tag="remT")
                nc.scalar.activation(remT16[:], remT_ps[:], AF.Copy)

                av2_ps = p3p.tile([128, R], dt.float32, tag="p3")
                au2_ps = p3p.tile([128, R], dt.float32, tag="p3")
                for fc in range(4):
                    nc.tensor.matmul(av2_ps[:], wv_sb[:, fc * D:(fc + 1) * D],
                                     remT16[:, fc * R:(fc + 1) * R],
                                     start=(fc == 0), stop=(fc == 3))
                for fc in range(4):
                    nc.tensor.matmul(au2_ps[:], wu_sb[:, fc * D:(fc + 1) * D],
                                     remT16[:, fc * R:(fc + 1) * R],
                                     start=(fc == 0), stop=(fc == 3))
                av2 = p3s.tile([128, R], dt.float32, tag="av2")
                au2 = p3s.tile([128, R], dt.float32, tag="au2")
                nc.scalar.activation(av2[:], av2_ps[:], AF.Tanh, bias=avb_sb[:])
                nc.scalar.activation(au2[:], au2_ps[:], AF.Sigmoid,
                                     bias=aub_sb[:])
                h2_16 = p3s.tile([128, R], dt.float16, tag="h2")
                nc.vector.tensor_mul(h2_16[:], av2[:], au2[:])

                w2_ps = p3p.tile([R, 1], dt.float32, tag="p3")
                nc.tensor.matmul(w2_ps[:], h2_16[:], aw_sb[:], start=True,
                                 stop=True, skip_group_check=True)
                w2_sb = p3s.tile([R, 1], dt.float32, tag="w2")
                nc.vector.tensor_scalar_add(w2_sb[:], w2_ps[:],
                                            awb_sb[0:R, :])
                # softmax over the 64 regions (transpose to a row first)
                w2T_ps = p3p.tile([1, R], dt.float32, tag="p3")
                nc.tensor.matmul(w2T_ps[:], w2_sb[:], ident_sb[0:R, 0:R],
                                 is_transpose=True, start=True, stop=True,
                                 skip_group_check=True)
                w2T = p3s.tile([1, R], dt.float32, tag="w2T")
                nc.vector.tensor_copy(w2T[:], w2T_ps[:])
                mr = p3s.tile([1, 1], dt.float32, tag="mr")
                nc.vector.reduce_max(mr[:], w2T[:], axis=mybir.AxisListType.X)
                negm = p3s.tile([1, 1], dt.float32, tag="negm")
                nc.vector.tensor_scalar_mul(negm[:], mr[:], -1.0)
                er = p3s.tile([1, R], dt.float32, tag="er")
                es = p3s.tile([1, 1], dt.float32, tag="es")
                nc.scalar.activation(er[:], w2T[:], AF.Exp, bias=negm[:],
                                     accum_out=es[:])
                rs = p3s.tile([1, 1], dt.float32, tag="rs")
                nc.vector.reciprocal(rs[:], es[:])
                smr = p3s.tile([1, R], dt.float32, tag="smr")
                nc.vector.tensor_scalar_mul(smr[:], er[:], rs[:])
                smrT_ps = p3p.tile([R, 1], dt.float32, tag="p3")
                nc.tensor.matmul(smrT_ps[:], smr[:], ident_sb[0:1, 0:1],
                                 is_transpose=True, start=True, stop=True,
                                 skip_group_check=True)
                smrT = p3s.tile([R, 1], dt.float32, tag="smrT")
                nc.vector.tensor_copy(smrT[:], smrT_ps[:])

                # embedding^T [512] as 4 psum columns: rem^T @ smr
                embT_ps = p3p.tile([128, 4], dt.float32, tag="p3")
                for fc in range(4):
                    nc.tensor.matmul(embT_ps[:, fc:fc + 1],
                                     rem_sb[:, fc * 128:(fc + 1) * 128],
                                     smrT[:], start=True, stop=True,
                                     skip_group_check=True)
                embT = p3s.tile([128, 4], dt.float32, tag="embT")
                nc.scalar.activation(embT[:], embT_ps[:], AF.Copy)

                # classifier MLP
                h1_ps = p3p.tile([128, 2], dt.float32, tag="p3")
                for mc in range(2):
                    for fc in range(4):
                        nc.tensor.matmul(
                            h1_ps[:, mc:mc + 1],
                            c1w_sb[:, fc * 256 + mc * 128:
                                   fc * 256 + (mc + 1) * 128],
                            embT[:, fc:fc + 1],
                            start=(fc == 0), stop=(fc == 3),
                            skip_group_check=True)
                h1 = p3s.tile([128, 2], dt.float32, tag="h1")
                for mc in range(2):
                    nc.scalar.activation(h1[:, mc:mc + 1], h1_ps[:, mc:mc + 1],
                                         AF.Relu, bias=c1b_sb[:, mc:mc + 1])
                h2_ps = p3p.tile([128, 1], dt.float32, tag="p3")
                for mc in range(2):
                    nc.tensor.matmul(h2_ps[:], c2w_sb[:, mc * D:(mc + 1) * D],
                                     h1[:, mc:mc + 1], start=(mc == 0),
                                     stop=(mc == 1), skip_group_check=True)
                h2 = p3s.tile([128, 1], dt.float32, tag="h2s")
                nc.scalar.activation(h2[:], h2_ps[:], AF.Relu, bias=c2b_sb[:])
                g_ps = p3p.tile([2, 1], dt.float32, tag="p3")
                nc.tensor.matmul(g_ps[:], c3w_sb[:], h2[:], start=True,
                                 stop=True, skip_group_check=True)
                g_sb = p3s.tile([2, 1], dt.float32, tag="gsb")
                nc.vector.tensor_scalar_add(g_sb[:], g_ps[:], c3b_sb[:])
                nc.sync.dma_start(glob[:], g_sb[:])

    nc.compile()
    return nc


def kernel(features, av_w, av_b, au_w, au_b, aw_w, aw_b,
           c1_w, c1_b, c2_w, c2_b, c3_w, c3_b,
           region_info, num_regions):
    assert int(num_regions) == R
    features = np.ascontiguousarray(np.asarray(features, dtype=np.float32))
    region_info = np.asarray(region_info)
    assert features.shape == (N, L) and region_info.shape == (N,)

    if "nc" not in _CACHE:
        _CACHE["nc"] = _build()
    nc = _CACHE["nc"]

    counts = np.bincount(region_info, minlength=R).astype(np.float32)
    iota_h = np.broadcast_to(np.arange(128, dtype=np.float32),
                             (128, 128)).copy()
    cnt_h = np.broadcast_to(counts, (128, R)).copy()
    ident_h = np.eye(128, dtype=np.float32)
    f16 = np.float16

    common = dict(
        iota=iota_h, cntb=cnt_h, ident=ident_h,
        wv=np.ascontiguousarray(av_w, dtype=f16),
        wu=np.ascontiguousarray(au_w, dtype=f16),
        aw16=np.ascontiguousarray(aw_w, dtype=f16),
        avb=np.asarray(av_b, np.float32).reshape(D, 1).copy(),
        aub=np.asarray(au_b, np.float32).reshape(D, 1).copy(),
        awb=np.full((128, 1), np.float32(np.asarray(aw_b).reshape(())),
                    np.float32),
        ones16=np.ones((128, 1), f16),
        c1w=np.ascontiguousarray(c1_w, np.float32),
        c1b=np.asarray(c1_b, np.float32).reshape(2, 128).T.copy(),
        c2w=np.ascontiguousarray(c2_w, np.float32),
        c2b=np.asarray(c2_b, np.float32).reshape(128, 1).copy(),
        c3w=np.ascontiguousarray(c3_w, np.float32),
        c3b=np.asarray(c3_b, np.float32).reshape(2, 1).copy(),
    )

    in_maps = []
    for c in range(NCORES):
        sl = slice(c * NL, (c + 1) * NL)
        xs = features[sl]
        segl = region_info[sl].astype(np.float32)
        m = dict(common)
        m["xn"] = xs.astype(f16)
        m["xt"] = np.ascontiguousarray(xs.T).astype(f16)
        m["segt"] = np.ascontiguousarray(segl.reshape(NT, 128).T)
        in_maps.append(m)

    res = run_bass_kernel_spmd(nc, in_maps, core_ids=list(range(NCORES)),
                               **_RUN_KWARGS)
    _CACHE["last_results"] = res

    patch = np.concatenate(
        [res.results[c]["patch_t"].T.reshape(-1) for c in range(NCORES)])
    glob_out = res.results[0]["glob"][:, 0].copy()
    return glob_out, patch[:, None].astype(np.float32)


# revision 13
# speedup vs baseline: 1.1378x; 1.1378x over previous
"""Trainium2 Bass kernel for NMIL (nested multiple-instance learning) architecture.

Computation (see reference): gated-attention MIL logits per instance, per-region
(segment) softmax + attention-pooled region embeddings, second-level attention
over regions, classifier MLP.

Distribution: instance dim N=131072 sharded across 8 cores (16384 each).
Host pre-transposes/casts the feature shard to f16 twice (natural [Nl,512] and
transposed [512,Nl]) so the device never transposes the big tensor; segment
softmax/sum partials are AllReduced across cores in-kernel; every core
redundantly computes the tiny level-2 tail; host takes core 0's result.

f16 is used for the large GEMM operands (X, Wv, Wu, one-hot*e); all
accumulation is f32 in PSUM. Verified end-to-end rel err ~2e-4 vs f32 ref.
"""

import numpy as np

import concourse.bacc as bacc
import concourse.mybir as mybir
import concourse.tile as tile
from concourse.bass_utils import run_bass_kernel_spmd

dt = mybir.dt
AF = mybir.ActivationFunctionType
OP = mybir.AluOpType

NCORES = 8
N, L, D, R = 131072, 512, 128, 64
NL = N // NCORES           # 16384 instances per core
NT = NL // 128             # 128 chunks of 128 instances
MACRO = 4                  # chunks per macro-tile (512 instances)
NMACRO = NT // MACRO       # 32

_RUN_KWARGS: dict = {}     # test.py may set dict(trace=True, ...)
_CACHE: dict = {}


def _build():
    nc = bacc.Bacc("TRN2", target_bir_lowering=False, debug=False,
                   num_devices=NCORES)

    def din(name, shape, dtype):
        return nc.dram_tensor(name, shape, dtype, kind="ExternalInput").ap()

    xt = din("xt", [L, NL], dt.float16)          # transposed feature shard
    xn = din("xn", [NL, L], dt.float16)          # natural feature shard
    segt = din("segt", [128, NT], dt.float32)    # tiled seg ids: [p,j]=seg[j*128+p]
    segb8 = din("segb8", [128, NT * R], dt.float16)  # seg_bc: col j*64+r = segt[p,j]
    iota = din("iota", [128, 128], dt.float32)   # each row = 0..127
    iota8 = din("iota8", [128, 8 * R], dt.float16)   # 8 tiled copies of 0..63
    cnt8 = din("cnt8", [128, 8 * R], dt.float32)     # 8 tiled copies of counts
    wv = din("wv", [L, D], dt.float16)
    wu = din("wu", [L, D], dt.float16)
    aw16 = din("aw16", [D, 1], dt.float16)
    avb = din("avb", [D, 1], dt.float32)
    aub = din("aub", [D, 1], dt.float32)
    awb = din("awb", [128, 1], dt.float32)       # aw_b replicated
    ones16 = din("ones16", [128, 1], dt.float16)
    ident = din("ident", [128, 128], dt.float32)
    c1w = din("c1w", [L, 256], dt.float32)
    c1b = din("c1b", [128, 2], dt.float32)
    c2w = din("c2w", [256, D], dt.float32)
    c2b = din("c2b", [128, 1], dt.float32)
    c3w = din("c3w", [D, 2], dt.float32)
    c3b = din("c3b", [2, 1], dt.float32)

    patch_t = nc.dram_tensor("patch_t", [128, NT], dt.float32,
                             kind="ExternalOutput").ap()
    glob = nc.dram_tensor("glob", [2, 1], dt.float32,
                          kind="ExternalOutput").ap()

    with tile.TileContext(nc) as tc:
        with (
            tc.tile_pool(name="const", bufs=1) as cp,
            tc.tile_pool(name="res", bufs=1) as rp,
            tc.tile_pool(name="dram", bufs=1, space="DRAM") as dp,
        ):
            # ---- load constants ----
            iota_sb = cp.tile([128, 128], dt.float32)
            nc.sync.dma_start(iota_sb[:], iota[:])
            iota8_sb = cp.tile([128, 8 * R], dt.float16)
            nc.sync.dma_start(iota8_sb[:], iota8[:])
            cnt8_sb = cp.tile([128, 8 * R], dt.float32)
            nc.sync.dma_start(cnt8_sb[:], cnt8[:])
            segb8_sb = cp.tile([128, NT * R], dt.float16)
            nc.sync.dma_start(segb8_sb[:], segb8[:])
            wv_sb = cp.tile([128, 4 * D], dt.float16)
            wu_sb = cp.tile([128, 4 * D], dt.float16)
            for fc in range(4):
                nc.sync.dma_start(wv_sb[:, fc * D:(fc + 1) * D],
                                  wv[fc * 128:(fc + 1) * 128, :])
                nc.sync.dma_start(wu_sb[:, fc * D:(fc + 1) * D],
                                  wu[fc * 128:(fc + 1) * 128, :])
            aw_sb = cp.tile([D, 1], dt.float16)
            nc.sync.dma_start(aw_sb[:], aw16[:])
            avb_sb = cp.tile([D, 1], dt.float32)
            nc.sync.dma_start(avb_sb[:], avb[:])
            aub_sb = cp.tile([D, 1], dt.float32)
            nc.sync.dma_start(aub_sb[:], aub[:])
            awb_sb = cp.tile([128, 1], dt.float32)
            nc.sync.dma_start(awb_sb[:], awb[:])
            ones_sb = cp.tile([128, 1], dt.float16)
            nc.sync.dma_start(ones_sb[:], ones16[:])
            ident_sb = cp.tile([128, 128], dt.float32)
            nc.sync.dma_start(ident_sb[:], ident[:])
            c1w_sb = cp.tile([128, 4 * 256], dt.float32)
            for fc in range(4):
                nc.sync.dma_start(c1w_sb[:, fc * 256:(fc + 1) * 256],
                                  c1w[fc * 128:(fc + 1) * 128, :])
            c1b_sb = cp.tile([128, 2], dt.float32)
            nc.sync.dma_start(c1b_sb[:], c1b[:])
            c2w_sb = cp.tile([128, 2 * D], dt.float32)
            for mc in range(2):
                nc.sync.dma_start(c2w_sb[:, mc * D:(mc + 1) * D],
                                  c2w[mc * 128:(mc + 1) * 128, :])
            c2b_sb = cp.tile([128, 1], dt.float32)
            nc.sync.dma_start(c2b_sb[:], c2b[:])
            c3w_sb = cp.tile([D, 2], dt.float32)
            nc.sync.dma_start(c3w_sb[:], c3w[:])
            c3b_sb = cp.tile([2, 1], dt.float32)
            nc.sync.dma_start(c3b_sb[:], c3b[:])

            # ---- resident buffers ----
            seg_all = rp.tile([128, NT], dt.float32)
            nc.sync.dma_start(seg_all[:], segt[:])
            w_all = rp.tile([128, NT], dt.float32)
            e_all = rp.tile([128, NT], dt.float32)
            zi_all = rp.tile([128, NT], dt.float32)
            cg_all = rp.tile([128, NT], dt.float32)
            patch_sb = rp.tile([128, NT], dt.float32)
            rem_un = rp.tile([R, L + 1], dt.float32)   # unnormed region emb | z
            rem_g = rp.tile([R, L + 1], dt.float32)    # after AllReduce
            rem_sb = rp.tile([R, L], dt.float32)       # normalized region emb
            z_row = rp.tile([1, R], dt.float32)

            bounce_in = dp.tile([R, L + 1], dt.float32)
            bounce_out = dp.tile([R, L + 1], dt.float32)

            # ================= phase 1: per-instance MIL + partial segsums ===
            with (
                tc.tile_pool(name="xin", bufs=3) as xp,
                tc.tile_pool(name="mid", bufs=2) as mp,
                tc.tile_pool(name="oep", bufs=3) as op_,
                tc.tile_pool(name="psm", bufs=2, space="PSUM") as psm,
                tc.tile_pool(name="psw", bufs=2, space="PSUM") as psw,
                tc.tile_pool(name="psacc", bufs=1, space="PSUM") as psa,
            ):
                rem_ps = psa.tile([128, L], dt.float32, tag="rem")
                z_ps = psa.tile([128, 1], dt.float32, tag="z")
                xt4 = xt.rearrange("(c p) n -> p c n", c=4)
                xn4 = xn.rearrange("(m t p) n -> m p t n", t=4, p=128)
                for m in range(NMACRO):
                    i0 = m * 512
                    xtm = xp.tile([128, 4 * 512], dt.float16, tag="xt")
                    xnm = xp.tile([128, 4 * 512], dt.float16, tag="xn")
                    nc.sync.dma_start(
                        xtm.rearrange("p (c n) -> p c n", c=4),
                        xt4[:, :, i0:i0 + 512])
                    nc.sync.dma_start(
                        xnm.rearrange("p (t n) -> p t n", t=4),
                        xn4[m])

                    av_ps = psm.tile([128, 512], dt.float32, tag="av")
                    au_ps = psm.tile([128, 512], dt.float32, tag="au")
                    for fc in range(4):
                        nc.tensor.matmul(
                            av_ps[:], wv_sb[:, fc * D:(fc + 1) * D],
                            xtm[:, fc * 512:(fc + 1) * 512],
                            start=(fc == 0), stop=(fc == 3))
                    for fc in range(4):
                        nc.tensor.matmul(
                            au_ps[:], wu_sb[:, fc * D:(fc + 1) * D],
                            xtm[:, fc * 512:(fc + 1) * 512],
                            start=(fc == 0), stop=(fc == 3))

                    # sigmoid(x) = (tanh(x/2)+1)/2: tu=tanh(0.5*(pu+b)) keeps
                    # everything in the exp_and_others table set (no swaps);
                    # the 0.5 factors are folded into aub (host) and aw (host)
                    av_sb = mp.tile([128, 512], dt.float32, tag="avs")
                    au_sb = mp.tile([128, 512], dt.float32, tag="aus")
                    nc.scalar.activation(av_sb[:], av_ps[:], AF.Tanh,
                                         bias=avb_sb[:])
                    nc.scalar.activation(au_sb[:], au_ps[:], AF.Tanh,
                                         bias=aub_sb[:], scale=0.5)
                    # h' = tv*(tu+1) = 2*av*sigmoid; aw was pre-halved
                    h16 = mp.tile([128, 512], dt.float16, tag="h")
                    nc.vector.scalar_tensor_tensor(
                        h16[:], au_sb[:], 1.0, av_sb[:],
                        op0=OP.add, op1=OP.mult)

                    w_ps = psw.tile([128, 4], dt.float32, tag="w")
                    for t in range(4):
                        nc.tensor.matmul(w_ps[:, t:t + 1],
                                         h16[:, t * 128:(t + 1) * 128],
                                         aw_sb[:], start=True, stop=True,
                                         skip_group_check=True)
                    c4 = slice(m * 4, (m + 1) * 4)
                    nc.scalar.activation(e_all[:, c4], w_ps[:], AF.Exp,
                                         bias=awb_sb[:])
                    nc.vector.tensor_scalar_add(w_all[:, c4], w_ps[:],
                                                awb_sb[:])

                    for t in range(4):
                        j = m * 4 + t
                        oe = op_.tile([128, 128], dt.float16, tag="oe")
                        nc.vector.tensor_scalar(
                            oe[:], iota_sb[:], seg_all[:, j:j + 1],
                            e_all[:, j:j + 1], op0=OP.is_equal, op1=OP.mult)
                        first = (m == 0 and t == 0)
                        last = (m == NMACRO - 1 and t == 3)
                        nc.tensor.matmul(rem_ps[:], oe[:],
                                         xnm[:, t * 512:(t + 1) * 512],
                                         start=first, stop=last,
                                         skip_group_check=True)
                        nc.tensor.matmul(z_ps[:], oe[:], ones_sb[:],
                                         start=first, stop=last,
                                         skip_group_check=True)

                nc.scalar.activation(rem_un[:, 0:L], rem_ps[0:R, :], AF.Copy)
                nc.scalar.activation(rem_un[:, L:L + 1], z_ps[0:R, :], AF.Copy)

            # ================= AllReduce partials across the 8 cores =========
            nc.sync.dma_start(bounce_in[:], rem_un[:])
            nc.gpsimd.collective_compute(
                "AllReduce", OP.add,
                replica_groups=[list(range(NCORES))],
                ins=[bounce_in.opt()], outs=[bounce_out.opt()])
            nc.sync.dma_start(rem_g[:], bounce_out[:])

            # ================= phase 2: per-instance softmax + patch out =====
            # zi[p,j] = z[seg[p,j]] and cg[p,j] = counts[seg[p,j]] gathers,
            # batched 8 chunks per op via one-hot * table + grouped reduce.
            # The counts path and one-hot build don't depend on the
            # AllReduce, so the scheduler can run them during it.
            with (
                tc.tile_pool(name="p2o", bufs=1) as p2o,
                tc.tile_pool(name="p2s", bufs=2) as p2s,
                tc.tile_pool(name="p2p", bufs=2, space="PSUM") as p2p,
            ):
                o_all = p2o.tile([128, NT * R], dt.float32, tag="oall")
                GR = 8 * R  # 512 columns per 8-chunk group
                for g in range(NT // 8):
                    nc.vector.tensor_tensor(
                        o_all[:, g * GR:(g + 1) * GR],
                        segb8_sb[:, g * GR:(g + 1) * GR], iota8_sb[:],
                        op=OP.is_equal)
                    cgp = p2s.tile([128, GR], dt.float32, tag="cgp")
                    nc.vector.tensor_tensor(
                        cgp[:], o_all[:, g * GR:(g + 1) * GR], cnt8_sb[:],
                        op=OP.mult)
                    nc.vector.tensor_reduce(
                        cg_all[:, g * 8:(g + 1) * 8],
                        cgp.rearrange("p (t r) -> p t r", r=R),
                        axis=mybir.AxisListType.X, op=OP.add)
                mask_all = p2s.tile([128, NT], dt.int32, tag="mask")
                nc.vector.tensor_scalar(mask_all[:], cg_all[:], 1.0, None,
                                        op0=OP.is_equal)

                # z row (depends on AR), broadcast via K=1 matmul, tiled 8x
                nc.sync.dma_start(
                    z_row[0:1, :],
                    bounce_out[:, L:L + 1].rearrange("a b -> b a"))
                z_row8 = p2s.tile([1, 8 * R], dt.float32, tag="zr8")
                for t in range(8):
                    nc.vector.tensor_copy(z_row8[0:1, t * R:(t + 1) * R],
                                          z_row[0:1, :])
                onecol = p2s.tile([1, 128], dt.float32, tag="onec")
                nc.gpsimd.memset(onecol[:], 1.0)
                zbc_ps = p2p.tile([128, 8 * R], dt.float32, tag="zbc")
                nc.tensor.matmul(zbc_ps[:], onecol[:], z_row8[0:1, :],
                                 start=True, stop=True, skip_group_check=True)
                zbc8 = p2s.tile([128, 8 * R], dt.float32, tag="zbc8")
                nc.scalar.activation(zbc8[:], zbc_ps[:], AF.Copy)

                for g in range(NT // 8):
                    zp = p2s.tile([128, GR], dt.float32, tag="zp")
                    nc.vector.tensor_tensor(
                        zp[:], o_all[:, g * GR:(g + 1) * GR], zbc8[:],
                        op=OP.mult)
                    nc.vector.tensor_reduce(
                        zi_all[:, g * 8:(g + 1) * 8],
                        zp.rearrange("p (t r) -> p t r", r=R),
                        axis=mybir.AxisListType.X, op=OP.add)

                rz_all = p2s.tile([128, NT], dt.float32, tag="rz")
                nc.vector.reciprocal(rz_all[:], zi_all[:])
                sm_all = p2s.tile([128, NT], dt.float32, tag="sm")
                nc.vector.tensor_mul(sm_all[:], e_all[:], rz_all[:])
                nc.vector.select(patch_sb[:], mask_all[:], w_all[:],
                                 sm_all[:])
                nc.sync.dma_start(patch_t[:], patch_sb[:])

            # ================= phase 3: level-2 attention + classifier =======
            with (
                tc.tile_pool(name="p3s", bufs=1) as p3s,
                tc.tile_pool(name="p3p", bufs=4, space="PSUM") as p3p,
            ):
                # rem = rem_g[:, :L] / z
                rzg = p3s.tile([R, 1], dt.float32, tag="rzg")
                nc.vector.reciprocal(rzg[:], rem_g[:, L:L + 1])
                nc.vector.tensor_scalar_mul(rem_sb[:], rem_g[:, 0:L], rzg[:])

                # remT via PE transposes (f32 in, cast to f16 on copy-out)
                remT_ps = p3p.tile([128, 4 * R], dt.float32, tag="p3")
                for fc in range(4):
                    nc.tensor.matmul(remT_ps[:, fc * R:(fc + 1) * R],
                                     rem_sb[:, fc * 128:(fc + 1) * 128],
                                     ident_sb[0:R, 0:R], is_transpose=True,
                                     start=True, stop=True,
                                     skip_group_check=True)
                remT16 = p3s.tile([128, 4 * R], dt.float16, tag="remT")
                nc.scalar.activation(remT16[:], remT_ps[:], AF.Copy)

                av2_ps = p3p.tile([128, R], dt.float32, tag="p3")
                au2_ps = p3p.tile([128, R], dt.float32, tag="p3")
                for fc in range(4):
                    nc.tensor.matmul(av2_ps[:], wv_sb[:, fc * D:(fc + 1) * D],
                                     remT16[:, fc * R:(fc + 1) * R],
                                     start=(fc == 0), stop=(fc == 3))
                for fc in range(4):
                    nc.tensor.matmul(au2_ps[:], wu_sb[:, fc * D:(fc + 1) * D],
                                     remT16[:, fc * R:(fc + 1) * R],
                                     start=(fc == 0), stop=(fc == 3))
                av2 = p3s.tile([128, R], dt.float32, tag="av2")
                au2 = p3s.tile([128, R], dt.float32, tag="au2")
                nc.scalar.activation(av2[:], av2_ps[:], AF.Tanh, bias=avb_sb[:])
                nc.scalar.activation(au2[:], au2_ps[:], AF.Tanh,
                                     bias=aub_sb[:], scale=0.5)
                h2_16 = p3s.tile([128, R], dt.float16, tag="h2")
                nc.vector.scalar_tensor_tensor(
                    h2_16[:], au2[:], 1.0, av2[:], op0=OP.add, op1=OP.mult)

                w2_ps = p3p.tile([R, 1], dt.float32, tag="p3")
                nc.tensor.matmul(w2_ps[:], h2_16[:], aw_sb[:], start=True,
                                 stop=True, skip_group_check=True)
                w2_sb = p3s.tile([R, 1], dt.float32, tag="w2")
                nc.vector.tensor_scalar_add(w2_sb[:], w2_ps[:],
                                            awb_sb[0:R, :])
                # softmax over the 64 regions (transpose to a row first)
                w2T_ps = p3p.tile([1, R], dt.float32, tag="p3")
                nc.tensor.matmul(w2T_ps[:], w2_sb[:], ident_sb[0:R, 0:R],
                                 is_transpose=True, start=True, stop=True,
                                 skip_group_check=True)
                w2T = p3s.tile([1, R], dt.float32, tag="w2T")
                nc.vector.tensor_copy(w2T[:], w2T_ps[:])
                mr = p3s.tile([1, 1], dt.float32, tag="mr")
                nc.vector.reduce_max(mr[:], w2T[:], axis=mybir.AxisListType.X)
                negm = p3s.tile([1, 1], dt.float32, tag="negm")
                nc.vector.tensor_scalar_mul(negm[:], mr[:], -1.0)
                er = p3s.tile([1, R], dt.float32, tag="er")
                es = p3s.tile([1, 1], dt.float32, tag="es")
                nc.scalar.activation(er[:], w2T[:], AF.Exp, bias=negm[:],
                                     accum_out=es[:])
                rs = p3s.tile([1, 1], dt.float32, tag="rs")
                nc.vector.reciprocal(rs[:], es[:])
                smr = p3s.tile([1, R], dt.float32, tag="smr")
                nc.vector.tensor_scalar_mul(smr[:], er[:], rs[:])
                smrT_ps = p3p.tile([R, 1], dt.float32, tag="p3")
                nc.tensor.matmul(smrT_ps[:], smr[:], ident_sb[0:1, 0:1],
                                 is_transpose=True, start=True, stop=True,
                                 skip_group_check=True)
                smrT = p3s.tile([R, 1], dt.float32, tag="smrT")
                nc.vector.tensor_copy(smrT[:], smrT_ps[:])

                # embedding^T [512] as 4 psum columns: rem^T @ smr
                embT_ps = p3p.tile([128, 4], dt.float32, tag="p3")
                for fc in range(4):
                    nc.tensor.matmul(embT_ps[:, fc:fc + 1],
                                     rem_sb[:, fc * 128:(fc + 1) * 128],
                                     smrT[:], start=True, stop=True,
                                     skip_group_check=True)
                embT = p3s.tile([128, 4], dt.float32, tag="embT")
                nc.scalar.activation(embT[:], embT_ps[:], AF.Copy)

                # classifier MLP
                h1_ps = p3p.tile([128, 2], dt.float32, tag="p3")
                for mc in range(2):
                    for fc in range(4):
                        nc.tensor.matmul(
                            h1_ps[:, mc:mc + 1],
                            c1w_sb[:, fc * 256 + mc * 128:
                                   fc * 256 + (mc + 1) * 128],
                            embT[:, fc:fc + 1],
                            start=(fc == 0), stop=(fc == 3),
                            skip_group_check=True)
                h1 = p3s.tile([128, 2], dt.float32, tag="h1")
                for mc in range(2):
                    nc.scalar.activation(h1[:, mc:mc + 1], h1_ps[:, mc:mc + 1],
                                         AF.Relu, bias=c1b_sb[:, mc:mc + 1])
                h2_ps = p3p.tile([128, 1], dt.float32, tag="p3")
                for mc in range(2):
                    nc.tensor.matmul(h2_ps[:], c2w_sb[:, mc * D:(mc + 1) * D],
                                     h1[:, mc:mc + 1], start=(mc == 0),
                                     stop=(mc == 1), skip_group_check=True)
                h2 = p3s.tile([128, 1], dt.float32, tag="h2s")
                nc.scalar.activation(h2[:], h2_ps[:], AF.Relu, bias=c2b_sb[:])
                g_ps = p3p.tile([2, 1], dt.float32, tag="p3")
                nc.tensor.matmul(g_ps[:], c3w_sb[:], h2[:], start=True,
                                 stop=True, skip_group_check=True)
                g_sb = p3s.tile([2, 1], dt.float32, tag="gsb")
                nc.vector.tensor_scalar_add(g_sb[:], g_ps[:], c3b_sb[:])
                nc.sync.dma_start(glob[:], g_sb[:])

    nc.compile()
    return nc


def kernel(features, av_w, av_b, au_w, au_b, aw_w, aw_b,
           c1_w, c1_b, c2_w, c2_b, c3_w, c3_b,
           region_info, num_regions):
    assert int(num_regions) == R
    features = np.ascontiguousarray(np.asarray(features, dtype=np.float32))
    region_info = np.asarray(region_info)
    assert features.shape == (N, L) and region_info.shape == (N,)

    if "nc" not in _CACHE:
        _CACHE["nc"] = _build()
    nc = _CACHE["nc"]

    counts = np.bincount(region_info, minlength=R).astype(np.float32)
    iota_h = np.broadcast_to(np.arange(128, dtype=np.float32),
                             (128, 128)).copy()
    iota8_h = np.broadcast_to(np.tile(np.arange(R, dtype=np.float16), 8),
                              (128, 8 * R)).copy()
    cnt8_h = np.broadcast_to(np.tile(counts, 8), (128, 8 * R)).copy()
    ident_h = np.eye(128, dtype=np.float32)
    f16 = np.float16

    common = dict(
        iota=iota_h, iota8=iota8_h, cnt8=cnt8_h, ident=ident_h,
        wv=np.ascontiguousarray(av_w, dtype=f16),
        wu=np.ascontiguousarray(au_w, dtype=f16),
        # 0.5 folded in: h' = tanh(pv+bv) * (tanh(0.5*pu+0.5*bu)+1) = 2h
        aw16=np.ascontiguousarray(np.asarray(aw_w) * 0.5, dtype=f16),
        avb=np.asarray(av_b, np.float32).reshape(D, 1).copy(),
        aub=(np.asarray(au_b, np.float32) * 0.5).reshape(D, 1).copy(),
        awb=np.full((128, 1), np.float32(np.asarray(aw_b).reshape(())),
                    np.float32),
        ones16=np.ones((128, 1), f16),
        c1w=np.ascontiguousarray(c1_w, np.float32),
        c1b=np.asarray(c1_b, np.float32).reshape(2, 128).T.copy(),
        c2w=np.ascontiguousarray(c2_w, np.float32),
        c2b=np.asarray(c2_b, np.float32).reshape(128, 1).copy(),
        c3w=np.ascontiguousarray(c3_w, np.float32),
        c3b=np.asarray(c3_b, np.float32).reshape(2, 1).copy(),
    )

    in_maps = []
    for c in range(NCORES):
        sl = slice(c * NL, (c + 1) * NL)
        xs = features[sl]
        segl = region_info[sl].astype(np.float32)
        m = dict(common)
        m["xn"] = xs.astype(f16)
        m["xt"] = np.ascontiguousarray(xs.T).astype(f16)
        segt_h = np.ascontiguousarray(segl.reshape(NT, 128).T)
        m["segt"] = segt_h
        m["segb8"] = np.repeat(segt_h.astype(f16), R, axis=1)
        in_maps.append(m)

    res = run_bass_kernel_spmd(nc, in_maps, core_ids=list(range(NCORES)),
                               **_RUN_KWARGS)
    _CACHE["last_results"] = res

    patch = np.concatenate(
        [res.results[c]["patch_t"].T.reshape(-1) for c in range(NCORES)])
    glob_out = res.results[0]["glob"][:, 0].copy()
    return glob_out, patch[:, None].astype(np.float32)


# revision 19
# speedup vs baseline: 1.1643x; 1.0233x over previous
"""Trainium2 Bass kernel for NMIL (nested multiple-instance learning) architecture.

Computation (see reference): gated-attention MIL logits per instance, per-region
(segment) softmax + attention-pooled region embeddings, second-level attention
over regions, classifier MLP.

Distribution: instance dim N=131072 sharded across 8 cores (16384 each).
Host pre-transposes/casts the feature shard to f16 twice (natural [Nl,512] and
transposed [512,Nl]) so the device never transposes the big tensor; segment
softmax/sum partials are AllReduced across cores in-kernel; every core
redundantly computes the tiny level-2 tail; host takes core 0's result.

f16 is used for the large GEMM operands (X, Wv, Wu, one-hot*e); all
accumulation is f32 in PSUM. Verified end-to-end rel err ~2e-4 vs f32 ref.
"""

import numpy as np

import concourse.bacc as bacc
import concourse.mybir as mybir
import concourse.tile as tile
from concourse.bass_utils import run_bass_kernel_spmd

dt = mybir.dt
AF = mybir.ActivationFunctionType
OP = mybir.AluOpType

NCORES = 8
N, L, D, R = 131072, 512, 128, 64
NL = N // NCORES           # 16384 instances per core
NT = NL // 128             # 128 chunks of 128 instances
MACRO = 4                  # chunks per macro-tile (512 instances)
NMACRO = NT // MACRO       # 32

_RUN_KWARGS: dict = {}     # test.py may set dict(trace=True, ...)
_CACHE: dict = {}


def _build(awb_val: float):
    nc = bacc.Bacc("TRN2", target_bir_lowering=False, debug=False,
                   num_devices=NCORES)

    def din(name, shape, dtype):
        return nc.dram_tensor(name, shape, dtype, kind="ExternalInput").ap()

    xt = din("xt", [L, NL], dt.float16)          # transposed feature shard
    xn = din("xn", [NL, L], dt.float16)          # natural feature shard
    segt = din("segt", [128, NT], dt.float32)    # tiled seg ids: [p,j]=seg[j*128+p]
    segb8 = din("segb8", [128, NT * R], dt.float16)  # seg_bc: col j*64+r = segt[p,j]
    iota = din("iota", [128, 128], dt.float32)   # each row = 0..127
    iota8 = din("iota8", [128, 8 * R], dt.float16)   # 8 tiled copies of 0..63
    cnt8 = din("cnt8", [128, 8 * R], dt.float32)     # 8 tiled copies of counts
    wv = din("wv", [L, D], dt.float16)
    wu = din("wu", [L, D], dt.float16)
    aw16 = din("aw16", [D, 1], dt.float16)
    avb = din("avb", [D, 1], dt.float32)
    aub = din("aub", [D, 1], dt.float32)
    awb = din("awb", [128, 1], dt.float32)       # aw_b replicated
    ones16 = din("ones16", [128, 1], dt.float16)
    ident = din("ident", [128, 128], dt.float32)
    c1w = din("c1w", [L, 256], dt.float32)
    c1b = din("c1b", [128, 2], dt.float32)
    c2w = din("c2w", [256, D], dt.float32)
    c2b = din("c2b", [128, 1], dt.float32)
    c3w = din("c3w", [D, 2], dt.float32)
    c3b = din("c3b", [2, 1], dt.float32)

    patch_t = nc.dram_tensor("patch_t", [128, NT], dt.float32,
                             kind="ExternalOutput").ap()
    glob = nc.dram_tensor("glob", [2, 1], dt.float32,
                          kind="ExternalOutput").ap()

    with tile.TileContext(nc) as tc:
        with (
            tc.tile_pool(name="const", bufs=1) as cp,
            tc.tile_pool(name="res", bufs=1) as rp,
            tc.tile_pool(name="dram", bufs=1, space="DRAM") as dp,
        ):
            # ---- load constants ----
            iota_sb = cp.tile([128, 128], dt.float32)
            nc.sync.dma_start(iota_sb[:], iota[:])
            # phase-2 constants: SWDGE queue, off the critical Sync queue
            iota8_sb = cp.tile([128, 8 * R], dt.float16)
            nc.gpsimd.dma_start(iota8_sb[:], iota8[:])
            cnt8_sb = cp.tile([128, 8 * R], dt.float32)
            nc.gpsimd.dma_start(cnt8_sb[:], cnt8[:])
            segb8_sb = cp.tile([128, NT * R], dt.float16)
            nc.gpsimd.dma_start(segb8_sb[:], segb8[:])
            wv_sb = cp.tile([128, 4 * D], dt.float16)
            wu_sb = cp.tile([128, 4 * D], dt.float16)
            for fc in range(4):
                nc.sync.dma_start(wv_sb[:, fc * D:(fc + 1) * D],
                                  wv[fc * 128:(fc + 1) * 128, :])
                nc.sync.dma_start(wu_sb[:, fc * D:(fc + 1) * D],
                                  wu[fc * 128:(fc + 1) * 128, :])
            aw_sb = cp.tile([D, 1], dt.float16)
            nc.sync.dma_start(aw_sb[:], aw16[:])
            avb_sb = cp.tile([D, 1], dt.float32)
            nc.sync.dma_start(avb_sb[:], avb[:])
            aub_sb = cp.tile([D, 1], dt.float32)
            nc.sync.dma_start(aub_sb[:], aub[:])
            awb_sb = cp.tile([128, 1], dt.float32)
            nc.sync.dma_start(awb_sb[:], awb[:])
            ones_sb = cp.tile([128, 1], dt.float16)
            nc.sync.dma_start(ones_sb[:], ones16[:])
            # phase-3 constants: SWDGE queue
            ident_sb = cp.tile([128, 128], dt.float32)
            nc.gpsimd.dma_start(ident_sb[:], ident[:])
            c1w_sb = cp.tile([128, 4 * 256], dt.float32)
            nc.gpsimd.dma_start(
                c1w_sb.rearrange("p (c n) -> p c n", c=4),
                c1w.rearrange("(c p) n -> p c n", c=4))
            c1b_sb = cp.tile([128, 2], dt.float32)
            nc.gpsimd.dma_start(c1b_sb[:], c1b[:])
            c2w_sb = cp.tile([128, 2 * D], dt.float32)
            nc.gpsimd.dma_start(
                c2w_sb.rearrange("p (c n) -> p c n", c=2),
                c2w.rearrange("(c p) n -> p c n", c=2))
            c2b_sb = cp.tile([128, 1], dt.float32)
            nc.gpsimd.dma_start(c2b_sb[:], c2b[:])
            c3w_sb = cp.tile([D, 2], dt.float32)
            nc.gpsimd.dma_start(c3w_sb[:], c3w[:])
            c3b_sb = cp.tile([2, 1], dt.float32)
            nc.gpsimd.dma_start(c3b_sb[:], c3b[:])

            # ---- resident buffers ----
            seg_all = rp.tile([128, NT], dt.float32)
            nc.sync.dma_start(seg_all[:], segt[:])
            w_all = rp.tile([128, NT], dt.float32)
            e_all = rp.tile([128, NT], dt.float32)
            zi_all = rp.tile([128, NT], dt.float32)
            cg_all = rp.tile([128, NT], dt.float32)
            patch_sb = rp.tile([128, NT], dt.float32)
            rem_un = rp.tile([R, L + 1], dt.float32)   # unnormed region emb | z
            rem_g = rp.tile([R, L + 1], dt.float32)    # after AllReduce
            rem_sb = rp.tile([R, L], dt.float32)       # normalized region emb
            z_row = rp.tile([1, R], dt.float32)

            bounce_in = dp.tile([R, L + 1], dt.float32)
            bounce_out = dp.tile([R, L + 1], dt.float32)

            # ================= phase 1: per-instance MIL + partial segsums ===
            with (
                tc.tile_pool(name="xin", bufs=3) as xp,
                tc.tile_pool(name="mid", bufs=3) as mp,
                tc.tile_pool(name="oep", bufs=4) as op_,
                tc.tile_pool(name="psm", bufs=2, space="PSUM") as psm,
                tc.tile_pool(name="psw", bufs=2, space="PSUM") as psw,
                tc.tile_pool(name="psacc", bufs=1, space="PSUM") as psa,
            ):
                rem_ps = psa.tile([128, L], dt.float32, tag="rem")
                z_ps = psa.tile([128, 1], dt.float32, tag="z")
                xt4 = xt.rearrange("(c p) n -> p c n", c=4)
                xn4 = xn.rearrange("(m t p) n -> m p t n", t=4, p=128)
                for m in range(NMACRO):
                    i0 = m * 512
                    xtm = xp.tile([128, 4 * 512], dt.float16, tag="xt")
                    xnm = xp.tile([128, 4 * 512], dt.float16, tag="xn")
                    nc.sync.dma_start(
                        xtm.rearrange("p (c n) -> p c n", c=4),
                        xt4[:, :, i0:i0 + 512])
                    nc.sync.dma_start(
                        xnm.rearrange("p (t n) -> p t n", t=4),
                        xn4[m])

                    av_ps = psm.tile([128, 512], dt.float32, tag="av")
                    au_ps = psm.tile([128, 512], dt.float32, tag="au")
                    for fc in range(4):
                        nc.tensor.matmul(
                            av_ps[:], wv_sb[:, fc * D:(fc + 1) * D],
                            xtm[:, fc * 512:(fc + 1) * 512],
                            start=(fc == 0), stop=(fc == 3))
                    for fc in range(4):
                        nc.tensor.matmul(
                            au_ps[:], wu_sb[:, fc * D:(fc + 1) * D],
                            xtm[:, fc * 512:(fc + 1) * 512],
                            start=(fc == 0), stop=(fc == 3))

                    # sigmoid(x) = (tanh(x/2)+1)/2: tu=tanh(0.5*(pu+b)) keeps
                    # everything in the exp_and_others table set (no swaps);
                    # the 0.5 factors are folded into aub (host) and aw (host)
                    av_sb = mp.tile([128, 512], dt.float32, tag="avs")
                    au_sb = mp.tile([128, 512], dt.float32, tag="aus")
                    nc.scalar.activation(av_sb[:], av_ps[:], AF.Tanh,
                                         bias=avb_sb[:])
                    nc.scalar.activation(au_sb[:], au_ps[:], AF.Tanh,
                                         bias=aub_sb[:], scale=0.5)
                    # h' = tv*(tu+1) = 2*av*sigmoid; aw was pre-halved
                    h16 = mp.tile([128, 512], dt.float16, tag="h")
                    nc.vector.scalar_tensor_tensor(
                        h16[:], au_sb[:], 1.0, av_sb[:],
                        op0=OP.add, op1=OP.mult)

                    w_ps = psw.tile([128, 4], dt.float32, tag="w")
                    for t in range(4):
                        nc.tensor.matmul(w_ps[:, t:t + 1],
                                         h16[:, t * 128:(t + 1) * 128],
                                         aw_sb[:], start=True, stop=True,
                                         skip_group_check=True)
                    c4 = slice(m * 4, (m + 1) * 4)
                    nc.scalar.activation(e_all[:, c4], w_ps[:], AF.Exp,
                                         bias=awb_sb[:])
                    nc.scalar.activation(w_all[:, c4], w_ps[:], AF.Copy,
                                         bias=awb_val)

                    for t in range(4):
                        j = m * 4 + t
                        oe = op_.tile([128, 128], dt.float16, tag="oe")
                        nc.vector.tensor_scalar(
                            oe[:], iota_sb[:], seg_all[:, j:j + 1],
                            e_all[:, j:j + 1], op0=OP.is_equal, op1=OP.mult)
                        first = (m == 0 and t == 0)
                        last = (m == NMACRO - 1 and t == 3)
                        nc.tensor.matmul(rem_ps[:], oe[:],
                                         xnm[:, t * 512:(t + 1) * 512],
                                         start=first, stop=last,
                                         skip_group_check=True)
                        nc.tensor.matmul(z_ps[:], oe[:], ones_sb[:],
                                         start=first, stop=last,
                                         skip_group_check=True)

                nc.scalar.activation(rem_un[:, 0:L], rem_ps[0:R, :], AF.Copy)
                nc.scalar.activation(rem_un[:, L:L + 1], z_ps[0:R, :], AF.Copy)

            # ================= AllReduce partials across the 8 cores =========
            nc.sync.dma_start(bounce_in[:], rem_un[:])
            nc.gpsimd.collective_compute(
                "AllReduce", OP.add,
                replica_groups=[list(range(NCORES))],
                ins=[bounce_in.opt()], outs=[bounce_out.opt()])
            nc.sync.dma_start(rem_g[:], bounce_out[:])

            # ================= phase 2: per-instance softmax + patch out =====
            # zi[p,j] = z[seg[p,j]] and cg[p,j] = counts[seg[p,j]] gathers,
            # batched 8 chunks per op via one-hot * table + grouped reduce.
            # The counts path and one-hot build don't depend on the
            # AllReduce, so the scheduler can run them during it.
            with (
                tc.tile_pool(name="p2o", bufs=1) as p2o,
                tc.tile_pool(name="p2s", bufs=2) as p2s,
                tc.tile_pool(name="p2p", bufs=2, space="PSUM") as p2p,
            ):
                o_all = p2o.tile([128, NT * R], dt.float32, tag="oall")
                GR = 8 * R  # 512 columns per 8-chunk group
                for g in range(NT // 8):
                    nc.vector.tensor_tensor(
                        o_all[:, g * GR:(g + 1) * GR],
                        segb8_sb[:, g * GR:(g + 1) * GR], iota8_sb[:],
                        op=OP.is_equal)
                    cgp = p2s.tile([128, GR], dt.float32, tag="cgp")
                    nc.vector.tensor_tensor(
                        cgp[:], o_all[:, g * GR:(g + 1) * GR], cnt8_sb[:],
                        op=OP.mult)
                    nc.vector.tensor_reduce(
                        cg_all[:, g * 8:(g + 1) * 8],
                        cgp.rearrange("p (t r) -> p t r", r=R),
                        axis=mybir.AxisListType.X, op=OP.add)
                mask_all = p2s.tile([128, NT], dt.int32, tag="mask")
                nc.vector.tensor_scalar(mask_all[:], cg_all[:], 1.0, None,
                                        op0=OP.is_equal)

                # z row (depends on AR), broadcast via K=1 matmul, tiled 8x
                nc.sync.dma_start(
                    z_row[0:1, :],
                    bounce_out[:, L:L + 1].rearrange("a b -> b a"))
                z_row8 = p2s.tile([1, 8 * R], dt.float32, tag="zr8")
                for t in range(8):
                    nc.vector.tensor_copy(z_row8[0:1, t * R:(t + 1) * R],
                                          z_row[0:1, :])
                onecol = p2s.tile([1, 128], dt.float32, tag="onec")
                nc.gpsimd.memset(onecol[:], 1.0)
                zbc_ps = p2p.tile([128, 8 * R], dt.float32, tag="zbc")
                nc.tensor.matmul(zbc_ps[:], onecol[:], z_row8[0:1, :],
                                 start=True, stop=True, skip_group_check=True)
                zbc8 = p2s.tile([128, 8 * R], dt.float32, tag="zbc8")
                nc.scalar.activation(zbc8[:], zbc_ps[:], AF.Copy)

                for g in range(NT // 8):
                    zp = p2s.tile([128, GR], dt.float32, tag="zp")
                    nc.vector.tensor_tensor(
                        zp[:], o_all[:, g * GR:(g + 1) * GR], zbc8[:],
                        op=OP.mult)
                    nc.vector.tensor_reduce(
                        zi_all[:, g * 8:(g + 1) * 8],
                        zp.rearrange("p (t r) -> p t r", r=R),
                        axis=mybir.AxisListType.X, op=OP.add)

                rz_all = p2s.tile([128, NT], dt.float32, tag="rz")
                nc.vector.reciprocal(rz_all[:], zi_all[:])
                sm_all = p2s.tile([128, NT], dt.float32, tag="sm")
                nc.vector.tensor_mul(sm_all[:], e_all[:], rz_all[:])
                nc.vector.select(patch_sb[:], mask_all[:], w_all[:],
                                 sm_all[:])
                nc.sync.dma_start(patch_t[:], patch_sb[:])

            # ================= phase 3: level-2 attention + classifier =======
            with (
                tc.tile_pool(name="p3s", bufs=1) as p3s,
                tc.tile_pool(name="p3p", bufs=4, space="PSUM") as p3p,
            ):
                # rem = rem_g[:, :L] / z
                rzg = p3s.tile([R, 1], dt.float32, tag="rzg")
                nc.vector.reciprocal(rzg[:], rem_g[:, L:L + 1])
                nc.vector.tensor_scalar_mul(rem_sb[:], rem_g[:, 0:L], rzg[:])

                # remT via PE transposes (f32 in, cast to f16 on copy-out)
                remT_ps = p3p.tile([128, 4 * R], dt.float32, tag="p3")
                for fc in range(4):
                    nc.tensor.matmul(remT_ps[:, fc * R:(fc + 1) * R],
                                     rem_sb[:, fc * 128:(fc + 1) * 128],
                                     ident_sb[0:R, 0:R], is_transpose=True,
                                     start=True, stop=True,
                                     skip_group_check=True)
                remT16 = p3s.tile([128, 4 * R], dt.float16, tag="remT")
                nc.scalar.activation(remT16[:], remT_ps[:], AF.Copy)

                av2_ps = p3p.tile([128, R], dt.float32, tag="p3")
                au2_ps = p3p.tile([128, R], dt.float32, tag="p3")
                for fc in range(4):
                    nc.tensor.matmul(av2_ps[:], wv_sb[:, fc * D:(fc + 1) * D],
                                     remT16[:, fc * R:(fc + 1) * R],
                                     start=(fc == 0), stop=(fc == 3))
                for fc in range(4):
                    nc.tensor.matmul(au2_ps[:], wu_sb[:, fc * D:(fc + 1) * D],
                                     remT16[:, fc * R:(fc + 1) * R],
                                     start=(fc == 0), stop=(fc == 3))
                av2 = p3s.tile([128, R], dt.float32, tag="av2")
                au2 = p3s.tile([128, R], dt.float32, tag="au2")
                nc.scalar.activation(av2[:], av2_ps[:], AF.Tanh, bias=avb_sb[:])
                nc.scalar.activation(au2[:], au2_ps[:], AF.Tanh,
                                     bias=aub_sb[:], scale=0.5)
                h2_16 = p3s.tile([128, R], dt.float16, tag="h2")
                nc.vector.scalar_tensor_tensor(
                    h2_16[:], au2[:], 1.0, av2[:], op0=OP.add, op1=OP.mult)

                w2_ps = p3p.tile([R, 1], dt.float32, tag="p3")
                nc.tensor.matmul(w2_ps[:], h2_16[:], aw_sb[:], start=True,
                                 stop=True, skip_group_check=True)
                w2_sb = p3s.tile([R, 1], dt.float32, tag="w2")
                nc.vector.tensor_scalar_add(w2_sb[:], w2_ps[:],
                                            awb_sb[0:R, :])
                # softmax over the 64 regions (transpose to a row first)
                w2T_ps = p3p.tile([1, R], dt.float32, tag="p3")
                nc.tensor.matmul(w2T_ps[:], w2_sb[:], ident_sb[0:R, 0:R],
                                 is_transpose=True, start=True, stop=True,
                                 skip_group_check=True)
                w2T = p3s.tile([1, R], dt.float32, tag="w2T")
                nc.vector.tensor_copy(w2T[:], w2T_ps[:])
                mr = p3s.tile([1, 1], dt.float32, tag="mr")
                nc.vector.reduce_max(mr[:], w2T[:], axis=mybir.AxisListType.X)
                negm = p3s.tile([1, 1], dt.float32, tag="negm")
                nc.vector.tensor_scalar_mul(negm[:], mr[:], -1.0)
                er = p3s.tile([1, R], dt.float32, tag="er")
                es = p3s.tile([1, 1], dt.float32, tag="es")
                nc.scalar.activation(er[:], w2T[:], AF.Exp, bias=negm[:],
                                     accum_out=es[:])
                rs = p3s.tile([1, 1], dt.float32, tag="rs")
                nc.vector.reciprocal(rs[:], es[:])
                smr = p3s.tile([1, R], dt.float32, tag="smr")
                nc.vector.tensor_scalar_mul(smr[:], er[:], rs[:])
                smrT_ps = p3p.tile([R, 1], dt.float32, tag="p3")
                nc.tensor.matmul(smrT_ps[:], smr[:], ident_sb[0:1, 0:1],
                                 is_transpose=True, start=True, stop=True,
                                 skip_group_check=True)
                smrT = p3s.tile([R, 1], dt.float32, tag="smrT")
                nc.vector.tensor_copy(smrT[:], smrT_ps[:])

                # embedding^T [512] as 4 psum columns: rem^T @ smr
                embT_ps = p3p.tile([128, 4], dt.float32, tag="p3")
                for fc in range(4):
                    nc.tensor.matmul(embT_ps[:, fc:fc + 1],
                                     rem_sb[:, fc * 128:(fc + 1) * 128],
                                     smrT[:], start=True, stop=True,
                                     skip_group_check=True)
                embT = p3s.tile([128, 4], dt.float32, tag="embT")
                nc.scalar.activation(embT[:], embT_ps[:], AF.Copy)

                # classifier MLP
                h1_ps = p3p.tile([128, 2], dt.float32, tag="p3")
                for mc in range(2):
                    for fc in range(4):
                        nc.tensor.matmul(
                            h1_ps[:, mc:mc + 1],
                            c1w_sb[:, fc * 256 + mc * 128:
                                   fc * 256 + (mc + 1) * 128],
                            embT[:, fc:fc + 1],
                            start=(fc == 0), stop=(fc == 3),
                            skip_group_check=True)
                h1 = p3s.tile([128, 2], dt.float32, tag="h1")
                for mc in range(2):
                    nc.scalar.activation(h1[:, mc:mc + 1], h1_ps[:, mc:mc + 1],
                                         AF.Relu, bias=c1b_sb[:, mc:mc + 1])
                h2_ps = p3p.tile([128, 1], dt.float32, tag="p3")
                for mc in range(2):
                    nc.tensor.matmul(h2_ps[:], c2w_sb[:, mc * D:(mc + 1) * D],
                                     h1[:, mc:mc + 1], start=(mc == 0),
                                     stop=(mc == 1), skip_group_check=True)
                h2 = p3s.tile([128, 1], dt.float32, tag="h2s")
                nc.scalar.activation(h2[:], h2_ps[:], AF.Relu, bias=c2b_sb[:])
                g_ps = p3p.tile([2, 1], dt.float32, tag="p3")
                nc.tensor.matmul(g_ps[:], c3w_sb[:], h2[:], start=True,
                                 stop=True, skip_group_check=True)
                g_sb = p3s.tile([2, 1], dt.float32, tag="gsb")
                nc.vector.tensor_scalar_add(g_sb[:], g_ps[:], c3b_sb[:])
                nc.sync.dma_start(glob[:], g_sb[:])

    nc.compile()
    return nc


def kernel(features, av_w, av_b, au_w, au_b, aw_w, aw_b,
           c1_w, c1_b, c2_w, c2_b, c3_w, c3_b,
           region_info, num_regions):
    assert int(num_regions) == R
    features = np.ascontiguousarray(np.asarray(features, dtype=np.float32))
    region_info = np.asarray(region_info)
    assert features.shape == (N, L) and region_info.shape == (N,)

    awb_val = float(np.asarray(aw_b).reshape(()))
    if _CACHE.get("awb_val") != awb_val:
        _CACHE["nc"] = _build(awb_val)
        _CACHE["awb_val"] = awb_val
    nc = _CACHE["nc"]

    counts = np.bincount(region_info, minlength=R).astype(np.float32)
    iota_h = np.broadcast_to(np.arange(128, dtype=np.float32),
                             (128, 128)).copy()
    iota8_h = np.broadcast_to(np.tile(np.arange(R, dtype=np.float16), 8),
                              (128, 8 * R)).copy()
    cnt8_h = np.broadcast_to(np.tile(counts, 8), (128, 8 * R)).copy()
    ident_h = np.eye(128, dtype=np.float32)
    f16 = np.float16

    common = dict(
        iota=iota_h, iota8=iota8_h, cnt8=cnt8_h, ident=ident_h,
        wv=np.ascontiguousarray(av_w, dtype=f16),
        wu=np.ascontiguousarray(au_w, dtype=f16),
        # 0.5 folded in: h' = tanh(pv+bv) * (tanh(0.5*pu+0.5*bu)+1) = 2h
        aw16=np.ascontiguousarray(np.asarray(aw_w) * 0.5, dtype=f16),
        avb=np.asarray(av_b, np.float32).reshape(D, 1).copy(),
        aub=(np.asarray(au_b, np.float32) * 0.5).reshape(D, 1).copy(),
        awb=np.full((128, 1), np.float32(np.asarray(aw_b).reshape(())),
                    np.float32),
        ones16=np.ones((128, 1), f16),
        c1w=np.ascontiguousarray(c1_w, np.float32),
        c1b=np.asarray(c1_b, np.float32).reshape(2, 128).T.copy(),
        c2w=np.ascontiguousarray(c2_w, np.float32),
        c2b=np.asarray(c2_b, np.float32).reshape(128, 1).copy(),
        c3w=np.ascontiguousarray(c3_w, np.float32),
        c3b=np.asarray(c3_b, np.float32).reshape(2, 1).copy(),
    )

    in_maps = []
    for c in range(NCORES):
        sl = slice(c * NL, (c + 1) * NL)
        xs = features[sl]
        segl = region_info[sl].astype(np.float32)
        m = dict(common)
        m["xn"] = xs.astype(f16)
        m["xt"] = np.ascontiguousarray(xs.T).astype(f16)
        segt_h = np.ascontiguousarray(segl.reshape(NT, 128).T)
        m["segt"] = segt_h
        m["segb8"] = np.repeat(segt_h.astype(f16), R, axis=1)
        in_maps.append(m)

    res = run_bass_kernel_spmd(nc, in_maps, core_ids=list(range(NCORES)),
                               **_RUN_KWARGS)
    _CACHE["last_results"] = res

    patch = np.concatenate(
        [res.results[c]["patch_t"].T.reshape(-1) for c in range(NCORES)])
    glob_out = res.results[0]["glob"][:, 0].copy()
    return glob_out, patch[:, None].astype(np.float32)
